# revision 27
# baseline (speedup 1.0000x reference)
"""Multi-head attention with "restricted softmax" on 8 TRN2 NeuronCores.

Reference computation (per head):
    score = Q @ K.T / sqrt(D)                       # [S, S]
    attn  = exp(score) / (1 + sum_k exp(score))     # restricted softmax
            (mathematically identical to the max-clamped reference form)
    out   = attn @ V                                # [S, D]

Full problem: B=2, H=16, S=2048, D=64  ->  32 heads, 4 heads per core.

The ScalarEngine's exp is the hard floor (1 elem/cycle/lane @ 1.2 GHz,
(N+352)/1.2 ns per instruction), so everything is built to keep it
saturated with the widest ACTIVATEs PSUM allows:
  - Scores computed TRANSPOSED (S^T[k, q]) in [128, 512] units; THREE
    units share one ACTIVATE (N=1536 -> 1.025 ns/elem vs 1.12 at N=1024):
    6 PSUM banks double-buffered + 2 banks for the PV accumulator ring.
  - Scores matmuls contract K=128 with zero-padded rows 64..127: K=64
    streams at the same rate, but half-height weights only light up half
    the PE array and the HAM clock-gate then never opens (kernel stuck at
    1.2 GHz instead of 2.4). A chained dummy-matmul burst during the DMA
    ramp opens the gate before the pipeline starts.
  - PV uses lhsT=[V | 1] so PSUM row 64 accumulates sum_k exp (the
    softmax denominator) for free.
  - The epilogue (normalize + [d,q]->[q,d]) uses PE transposes (4 x
    ~215ns/pass fits the PE's per-group slack) instead of a DRAM round
    trip: DMA lateness can then never reach the critical DVE stream. The
    oT eviction cast is issued immediately at pass end; the one 4-wide
    tp allocation per pass keeps the ps_o ring parity even.
  - All emission is deferred via a unit-indexed action heap so every op's
    dependency is already satisfied at dispatch: the DVE / Sync / GpSimd
    queues are in-order and a waiting op head-of-line-blocks its stream.
  - Queue split: GpSimd carries staging loads/bounces + output DMAs,
    Sync carries only the staging X-bar transposes (HWDGE-only). Head
    staging is fully WAR-free (bufs=4 pools) and starts during the ramp.
  - Head 0 is staged by PE JIT transposes (the PE is idle in the ramp);
    heads 1..3 go through an fp16 DRAM bounce + X-bar transpose.
  - The exp table is pre-warmed so ACT_TABLE_LOAD overlaps the ramp.
"""

import heapq
import os

import numpy as np

import concourse.bass as bass  # noqa: F401  (bass must import before tile)
import concourse.mybir as mybir
import concourse.tile as tile
from concourse import bacc
from concourse.bass_utils import run_bass_kernel_spmd
from concourse.masks import make_identity

B, H, S, D = 2, 16, 2048, 64
N_CORES = 8
HPC = (B * H) // N_CORES  # heads per core = 4

F32 = mybir.dt.float32
F16 = mybir.dt.float16
EXP = mybir.ActivationFunctionType.Exp

SCALE = 1.0 / 8.0   # 1/sqrt(D)
NK = S // 128       # 16 k-tiles of 128
QB = 512            # q-block width per pass
NQB = S // QB       # 4 q-blocks per head
NPASS = HPC * NQB   # 16 passes
UNITS = NPASS * NK  # 256 scores units of [128k, 512q]
GRP = 3             # units per ACTIVATE group


class _HeadInputs:
    """Per-head staged inputs: fp16 Q^T/K^T [128, S] (rows 64..127 are
    zeros; scores contract K=128 to keep the PE array fully lit for the
    HAM activity monitor) and [V | 1] fp16.

    Heads 1..3: DMA X-bar transpose of an fp16 bounce buffer in DRAM
    (zero PE cost), with each stage emitted as a deferred action so no
    queue ever dispatches a waiting op. Head 0: PE transposes JIT'd into
    the idle ramp."""

    def __init__(self, ctx, h):
        self.ctx = ctx
        self.h = h
        self.ready = {"q": set(), "k": set()}  # head-0 JIT transpose state

    def _alloc(self):
        pools, h = self.ctx, self.h
        hp = pools["head_pool"]
        self.q_nat = hp.tile([128, NK, D], F32, tag="q_nat", name=f"q_nat{h}")
        self.k_nat = hp.tile([128, NK, D], F32, tag="k_nat", name=f"k_nat{h}")
        self.v_nat = hp.tile([128, NK, D], F32, tag="v_nat", name=f"v_nat{h}")
        # fp16 staging; cols 64..127 are never written nor read
        self.q16 = hp.tile([128, NK, 128], F16, tag="q16", name=f"q16_{h}")
        self.k16 = hp.tile([128, NK, 128], F16, tag="k16", name=f"k16_{h}")
        self.v1 = hp.tile([128, NK, D + 1], F16, tag="v1", name=f"v1_{h}")
        self.qT = pools["qkt_pool"].tile([128, S], F16, tag="qT", name=f"qT{h}")
        self.kT = pools["qkt_pool"].tile([128, S], F16, tag="kT", name=f"kT{h}")

    # ---- heads 1..3: four deferred stages ----
    def stage_nat(self):
        nc, pools = self.ctx["nc"], self.ctx
        for nat, src in (
            (self.k_nat, pools["k_dram"]), (self.q_nat, pools["q_dram"]),
            (self.v_nat, pools["v_dram"]),
        ):
            nc.gpsimd.dma_start(
                nat[:], src[self.h].rearrange("(n p) d -> p n d", p=128)
            )

    def stage_cast(self):
        nc, pools = self.ctx["nc"], self.ctx
        nc.vector.tensor_copy(self.k16[:, :, :D], self.k_nat[:])
        nc.vector.tensor_copy(self.q16[:, :, :D], self.q_nat[:])
        nc.vector.tensor_copy(
            self.v1[:, :, D:].rearrange("p n one -> p (n one)"), pools["ones"][:]
        )
        nc.vector.tensor_copy(self.v1[:, :, :D], self.v_nat[:])

    def stage_bounce(self):
        nc, pools, h = self.ctx["nc"], self.ctx, self.h
        dp = pools["dram_pool"]
        self.qdr = dp.tile([S, 128], F16, tag="qdr", name=f"qdr{h}")
        self.kdr = dp.tile([S, 128], F16, tag="kdr", name=f"kdr{h}")
        for dr, st16 in ((self.kdr, self.k16), (self.qdr, self.q16)):
            nc.gpsimd.dma_start(
                dr[:].rearrange("(n p) c -> p n c", p=128), st16[:]
            )

    def stage_xbar(self):
        nc, tc = self.ctx["nc"], self.ctx["tc"]
        with tc.high_priority():
            nc.sync.dma_start_transpose(self.kT[:], self.kdr[:])
            nc.sync.dma_start_transpose(self.qT[:], self.qdr[:])

    # ---- head 0: chunked ramp + JIT PE transposes ----
    def ramp_dma(self):
        nc, pools = self.ctx["nc"], self.ctx
        self._alloc()
        # zero qT/kT rows 64..127 (gpsimd, idle during the ramp): scores
        # contract K=128 against zero-padded weights because half-height
        # (K=64) matmuls only light up half the PE array and the HAM
        # clock-gate then never opens (1.2 GHz).
        nc.gpsimd.memset(self.qT[D:, :], 0.0)
        nc.gpsimd.memset(self.kT[D:, :], 0.0)
        chunks = ((0, 4), (4, 12))  # n-block ranges: small first chunk
        for n0, nn in chunks:
            ns = slice(n0, n0 + nn)
            for nat, st16, src in (
                (self.k_nat, self.k16, pools["k_dram"]),
                (self.q_nat, self.q16, pools["q_dram"]),
            ):
                nc.sync.dma_start(
                    nat[:, ns, :],
                    src[0].rearrange("(n p) d -> p n d", p=128)[:, ns, :],
                )
                nc.vector.tensor_copy(st16[:, ns, :D], nat[:, ns, :])
            if n0 == 0:
                nc.sync.dma_start(
                    self.v_nat[:],
                    pools["v_dram"][0].rearrange("(n p) d -> p n d", p=128),
                )
        nc.vector.tensor_copy(
            self.v1[:, :, D:].rearrange("p n one -> p (n one)"), pools["ones"][:]
        )
        nc.vector.tensor_copy(self.v1[:, :, :D], self.v_nat[:])
        # pre-transpose only what group 0 needs; the rest JIT lazily
        for n in (0, 1, 2):
            self.ensure_h0("k", n)
        for n in (0, 1, 2, 3):
            self.ensure_h0("q", n)

    def ensure_h0(self, kind, n):
        """JIT a [64, 128] PE transpose of staging block n into qT/kT."""
        if n in self.ready[kind]:
            return
        self.ready[kind].add(n)
        nc, pools = self.ctx["nc"], self.ctx
        st16, tT = (self.q16, self.qT) if kind == "q" else (self.k16, self.kT)
        tp = pools["ps_o_pool"].tile([D, 128], F16, tag="oT", name="tp")
        nc.tensor.transpose(tp[:], st16[:, n, :D], pools["ident16"][:])
        nc.vector.tensor_copy(tT[:D, n * 128:(n + 1) * 128], tp[:])


def _attention(tc):
    nc = tc.nc
    q_dram = nc.dram_tensor("query", [HPC, S, D], F32, kind="ExternalInput").ap()
    k_dram = nc.dram_tensor("key", [HPC, S, D], F32, kind="ExternalInput").ap()
    v_dram = nc.dram_tensor("value", [HPC, S, D], F32, kind="ExternalInput").ap()
    o_dram = nc.dram_tensor("out", [HPC, S, D], F32, kind="ExternalOutput").ap()

    with (
        tc.tile_pool(name="const", bufs=1) as const_pool,
        tc.tile_pool(name="head_io", bufs=4) as head_pool,
        tc.tile_pool(name="qkt", bufs=4) as qkt_pool,
        tc.tile_pool(name="et", bufs=3) as et_pool,
        tc.tile_pool(name="epi", bufs=4) as epi_pool,
        tc.tile_pool(name="dram", bufs=4, space="DRAM") as dram_pool,
        tc.tile_pool(name="ps_g", bufs=2, space="PSUM") as ps_g_pool,
        tc.tile_pool(name="ps_o", bufs=2, space="PSUM") as ps_o_pool,
    ):
        ones = const_pool.tile([128, NK], F16)
        nc.vector.memset(ones[:], 1.0)
        # pre-warm the exp table so ACT_TABLE_LOAD overlaps the DMA ramp
        warm = const_pool.tile([128, 1], F16)
        nc.vector.memset(warm[:], 0.0)
        nc.scalar.activation(warm[:], warm[:], EXP)
        ident16 = const_pool.tile([128, 128], F16)
        make_identity(nc, ident16[:])

        ctx = {
            "nc": nc, "tc": tc, "q_dram": q_dram, "k_dram": k_dram, "v_dram": v_dram,
            "head_pool": head_pool, "qkt_pool": qkt_pool,
            "dram_pool": dram_pool, "ps_o_pool": ps_o_pool,
            "ones": ones, "ident16": ident16,
        }

        # HAM warm-up: ~6.8us of chained dummy matmuls while the DMA ramp
        # runs. The PE clock-gate only opens (1.2 -> 2.4 GHz) after a
        # fully-busy free-running 3.4us window (worst case ~7us of
        # continuous activity), and the steady-state pipeline's micro-idles
        # never provide one; without this the whole kernel runs at half PE
        # clock and the PE becomes the bottleneck. The accumulation chain
        # keeps the Tile scheduler from scattering the matmuls.
        dummy = const_pool.tile([128, 640], F16)
        nc.vector.memset(dummy[:], 0.0)
        warm_ps = ps_g_pool.tile([128, GRP, QB], F32, tag="s", name="warm_ps")
        for i in range(12):
            nc.tensor.matmul(
                warm_ps[:, 0, :], dummy[:, :128], dummy[:, 128:],
                start=(i == 0), stop=(i == 11),
            )

        heads = [_HeadInputs(ctx, h) for h in range(HPC)]
        heads[0].ramp_dma()
        for hd in heads[1:]:
            hd._alloc()
        for hd in heads[1:]:
            # start ALL heads' input loads first (transfers overlap the
            # pad memsets), all on the idle gpsimd during the ramp
            hd.stage_nat()
        for hd in heads[1:]:
            nc.gpsimd.memset(hd.q16[:, :, D:], 0.0)
            nc.gpsimd.memset(hd.k16[:, :, D:], 0.0)

        # deferred-action scheduler, keyed by unit index
        actions = []
        aseq = [0]

        def defer(due_u, fn):
            heapq.heappush(actions, (due_u, aseq[0], fn))
            aseq[0] += 1

        def run_due(u):
            while actions and actions[0][0] <= u:
                heapq.heappop(actions)[2]()

        def emit_scores(units):
            slot = ps_g_pool.tile([128, GRP, QB], F32, tag="s", name="s")
            for j, u in enumerate(units):
                p, k = divmod(u, NK)
                h, qb = divmod(p, NQB)
                hd = heads[h]
                if h == 0:
                    hd.ensure_h0("k", k)
                    for nb in range(qb * 4, qb * 4 + 4):
                        hd.ensure_h0("q", nb)
                nc.tensor.matmul(
                    slot[:, j, :],
                    hd.kT[:, k * 128:(k + 1) * 128],
                    hd.qT[:, qb * QB:(qb + 1) * QB],
                    start=True, stop=True,
                )
            return slot

        def emit_epilogue(h, qb, oT16, u_end):
            """Normalize + un-transpose oT16 [65,512] -> out [512,64] using
            PE transposes (4x ~215ns fits in the PE's per-group slack): no
            DRAM round trip, so no DMA lateness can ever reach the critical
            DVE stream. The single 4-wide tp allocation keeps the ps_o ring
            parity even (oT and tp alternate slots)."""
            o_sb = epi_pool.tile([128, 4, D], F32, tag="o_sb", name="o_sb")
            tp_ref = []

            def stage_tp():
                tp = ps_o_pool.tile([128, 4, 68], F16, tag="oT", name="tp_epi")
                for j in range(4):
                    nc.tensor.transpose(
                        tp[:, j, :65], oT16[:, j * 128:(j + 1) * 128],
                        ident16[:65, :65],
                    )
                tp_ref.append(tp)

            def stage_d():
                tp = tp_ref[0]
                den = epi_pool.tile([128, 4], F32, tag="den", name="den")
                nc.vector.tensor_scalar_add(den[:], tp[:, :, D], 1.0)
                rec = epi_pool.tile([128, 4], F32, tag="rec", name="rec")
                nc.vector.reciprocal(rec[:], den[:])
                for j in range(4):
                    nc.vector.tensor_scalar_mul(
                        o_sb[:, j, :], tp[:, j, :D], rec[:, j:j + 1]
                    )

            def stage_e():
                nc.gpsimd.dma_start(
                    o_dram[h].rearrange("(n p) d -> p n d", p=128)[:, qb * 4:qb * 4 + 4, :],
                    o_sb[:],
                )

            defer(u_end + 2, stage_tp)
            defer(u_end + 4, stage_d)
            defer(u_end + 6, stage_e)

        # stage heads 1..3 early with wide margins: pools are deep enough
        # (bufs=4) that no slot WAR exists; loads already run from the ramp
        for h_n, base in ((1, 18), (2, 34), (3, 50)):
            defer(base + 0, heads[h_n].stage_cast)
            defer(base + 2, heads[h_n].stage_bounce)
            defer(base + 6, heads[h_n].stage_xbar)
        # pre-spread head-0's q-block JITs into PE slack ahead of each
        # pass boundary instead of a 4-wide burst at the boundary itself
        for qn, qbase in ((4, 8), (8, 24), (12, 40)):
            for i in range(4):
                defer(
                    qbase + i,
                    (lambda nn: lambda: heads[0].ensure_h0("q", nn))(qn + i),
                )

        groups = [
            list(range(gs, min(gs + GRP, UNITS))) for gs in range(0, UNITS, GRP)
        ]
        slot_cur = emit_scores(groups[0])
        oT = None
        for g, units in enumerate(groups):
            w = len(units)
            et = et_pool.tile([128, GRP, QB], F16, tag="et", name="et")
            nc.scalar.activation(
                et[:, :w, :], slot_cur[:, :w, :], EXP, scale=SCALE
            )
            if g + 1 < len(groups):
                slot_cur = emit_scores(groups[g + 1])
            for j, u in enumerate(units):
                run_due(u)
                p, k = divmod(u, NK)
                h, qb = divmod(p, NQB)
                if k == 0:
                    oT = ps_o_pool.tile([65, QB], F32, tag="oT", name="oT")
                nc.tensor.matmul(
                    oT[:], heads[h].v1[:, k, :], et[:, j, :],
                    start=(k == 0), stop=(k == NK - 1),
                )
                if k == NK - 1:
                    # evict oT immediately (fp16 cast on DVE): the next
                    # pass's oT shares the slot ping-pong and its first PV
                    # must not wait
                    oT16 = epi_pool.tile([65, QB], F16, tag="oT16", name="oT16")
                    nc.vector.tensor_copy(oT16[:], oT[:])
                    emit_epilogue(h, qb, oT16, u)
        while actions:
            heapq.heappop(actions)[2]()


_NC_CACHE = None
_TRACE_READY = False


def _enable_tracing():
    """Register the NTFF profile hook that this image's antenv lacks, and
    keep profiling artifacts local instead of uploading to a bucket."""
    global _TRACE_READY
    if _TRACE_READY:
        return
    import sys
    import types

    import antenv
    import concourse.bass_utils as bu
    from trn_agent_boot.trn_boot import _ntff_profile_via_ctypes

    if "antenv.axon_hooks" not in sys.modules:
        mod = types.ModuleType("antenv.axon_hooks")
        mod._hook = None

        def set_axon_ntff_profile_hook(h):
            mod._hook = h

        def get_axon_ntff_profile_hook():
            return mod._hook

        mod.set_axon_ntff_profile_hook = set_axon_ntff_profile_hook
        mod.get_axon_ntff_profile_hook = get_axon_ntff_profile_hook
        sys.modules["antenv.axon_hooks"] = mod
        antenv.axon_hooks = mod

    hooks = sys.modules["antenv.axon_hooks"]
    if hooks.get_axon_ntff_profile_hook() is None:
        hooks.set_axon_ntff_profile_hook(
            _ntff_profile_via_ctypes("/opt/axon/libaxon_pjrt.so")
        )
    bu.upload_artifacts = lambda tmpdir: tmpdir
    _TRACE_READY = True


def _build():
    global _NC_CACHE
    if _NC_CACHE is None:
        nc = bacc.Bacc("TRN2", target_bir_lowering=False, debug=False)
        with tile.TileContext(nc) as tc:
            _attention(tc)
        nc.compile()
        _NC_CACHE = nc
    return _NC_CACHE


def _run(query, key, value, trace=False, tmpdir=None):
    if trace:
        _enable_tracing()
    q = np.ascontiguousarray(np.asarray(query, dtype=np.float32).reshape(B * H, S, D))
    k = np.ascontiguousarray(np.asarray(key, dtype=np.float32).reshape(B * H, S, D))
    v = np.ascontiguousarray(np.asarray(value, dtype=np.float32).reshape(B * H, S, D))
    in_maps = [
        {
            "query": q[c * HPC:(c + 1) * HPC],
            "key": k[c * HPC:(c + 1) * HPC],
            "value": v[c * HPC:(c + 1) * HPC],
        }
        for c in range(N_CORES)
    ]
    nc = _build()
    res = run_bass_kernel_spmd(
        nc, in_maps, core_ids=list(range(N_CORES)), trace=trace, tmpdir=tmpdir
    )
    out = np.stack([res.results[c]["out"] for c in range(N_CORES)])  # [8, HPC, S, D]
    return out.reshape(B, H, S, D), res


def kernel(query, key, value):
    out, _ = _run(query, key, value, trace=bool(int(os.environ.get("BASS_TRACE", "0"))))
    return out


# revision 28
# speedup vs baseline: 1.0975x; 1.0975x over previous
"""Multi-head attention with "restricted softmax" on 8 TRN2 NeuronCores.

Reference computation (per head):
    score = Q @ K.T / sqrt(D)                       # [S, S]
    attn  = exp(score) / (1 + sum_k exp(score))     # restricted softmax
            (mathematically identical to the max-clamped reference form)
    out   = attn @ V                                # [S, D]

Full problem: B=2, H=16, S=2048, D=64  ->  32 heads, 4 heads per core.

The ScalarEngine's exp is the hard floor (1 elem/cycle/lane @ 1.2 GHz,
(N+352)/1.2 ns per instruction), so everything is built to keep it
saturated with the widest ACTIVATEs PSUM allows:
  - Scores computed TRANSPOSED (S^T[k, q]) in [128, 512] units; THREE
    units share one ACTIVATE (N=1536 -> 1.025 ns/elem vs 1.12 at N=1024):
    6 PSUM banks double-buffered + 2 banks for the PV accumulator ring.
  - Scores matmuls contract K=128 with zero-padded rows 64..127: K=64
    streams at the same rate, but half-height weights only light up half
    the PE array and the HAM clock-gate then never opens (kernel stuck at
    1.2 GHz instead of 2.4). A chained dummy-matmul burst during the DMA
    ramp opens the gate before the pipeline starts.
  - PV uses lhsT=[V | 1] so PSUM row 64 accumulates sum_k exp (the
    softmax denominator) for free.
  - The epilogue (normalize + [d,q]->[q,d]) uses PE transposes (4 x
    ~215ns/pass fits the PE's per-group slack) instead of a DRAM round
    trip: DMA lateness can then never reach the critical DVE stream. The
    oT eviction cast is issued immediately at pass end; the one 4-wide
    tp allocation per pass keeps the ps_o ring parity even.
  - All emission is deferred via a unit-indexed action heap so every op's
    dependency is already satisfied at dispatch: the DVE / Sync / GpSimd
    queues are in-order and a waiting op head-of-line-blocks its stream.
  - Queue split: GpSimd carries staging loads/bounces + output DMAs,
    Sync carries only the staging X-bar transposes (HWDGE-only). Head
    staging is fully WAR-free (bufs=4 pools) and starts during the ramp.
  - Head 0 is staged by PE JIT transposes (the PE is idle in the ramp);
    heads 1..3 go through an fp16 DRAM bounce + X-bar transpose.
  - The exp table is pre-warmed so ACT_TABLE_LOAD overlaps the ramp.
"""

import heapq
import os

import numpy as np

import concourse.bass as bass  # noqa: F401  (bass must import before tile)
import concourse.mybir as mybir
import concourse.tile as tile
from concourse import bacc
from concourse.bass_utils import run_bass_kernel_spmd
from concourse.masks import make_identity

B, H, S, D = 2, 16, 2048, 64
N_CORES = 8
HPC = (B * H) // N_CORES  # heads per core = 4

F32 = mybir.dt.float32
F16 = mybir.dt.float16
EXP = mybir.ActivationFunctionType.Exp

SCALE = 1.0 / 8.0   # 1/sqrt(D)
NK = S // 128       # 16 k-tiles of 128
QB = 512            # q-block width per pass
NQB = S // QB       # 4 q-blocks per head
NPASS = HPC * NQB   # 16 passes
UNITS = NPASS * NK  # 256 scores units of [128k, 512q]
GRP = 3             # units per ACTIVATE group


class _HeadInputs:
    """Per-head staged inputs: fp16 Q^T/K^T [128, S] (rows 64..127 are
    zeros; scores contract K=128 to keep the PE array fully lit for the
    HAM activity monitor) and [V | 1] fp16.

    Heads 1..3: DMA X-bar transpose of an fp16 bounce buffer in DRAM
    (zero PE cost), with each stage emitted as a deferred action so no
    queue ever dispatches a waiting op. Head 0: PE transposes JIT'd into
    the idle ramp."""

    def __init__(self, ctx, h):
        self.ctx = ctx
        self.h = h
        self.ready = {"q": set(), "k": set()}  # head-0 JIT transpose state

    def _alloc(self):
        pools, h = self.ctx, self.h
        hp = pools["head_pool"]
        self.q_nat = hp.tile([128, NK, D], F32, tag="q_nat", name=f"q_nat{h}")
        self.k_nat = hp.tile([128, NK, D], F32, tag="k_nat", name=f"k_nat{h}")
        self.v_nat = hp.tile([128, NK, D], F32, tag="v_nat", name=f"v_nat{h}")
        # fp16 staging; cols 64..127 are never written nor read
        self.q16 = hp.tile([128, NK, 128], F16, tag="q16", name=f"q16_{h}")
        self.k16 = hp.tile([128, NK, 128], F16, tag="k16", name=f"k16_{h}")
        self.v1 = hp.tile([128, NK, D + 1], F16, tag="v1", name=f"v1_{h}")
        self.qT = pools["qkt_pool"].tile([128, S], F16, tag="qT", name=f"qT{h}")
        self.kT = pools["qkt_pool"].tile([128, S], F16, tag="kT", name=f"kT{h}")

    # ---- heads 1..3: four deferred stages ----
    def stage_nat(self):
        nc, pools = self.ctx["nc"], self.ctx
        for nat, src in (
            (self.k_nat, pools["k_dram"]), (self.q_nat, pools["q_dram"]),
            (self.v_nat, pools["v_dram"]),
        ):
            nc.gpsimd.dma_start(
                nat[:], src[self.h].rearrange("(n p) d -> p n d", p=128)
            )

    def stage_cast(self):
        nc, pools = self.ctx["nc"], self.ctx
        nc.vector.tensor_copy(self.k16[:, :, :D], self.k_nat[:])
        nc.vector.tensor_copy(self.q16[:, :, :D], self.q_nat[:])
        nc.vector.tensor_copy(
            self.v1[:, :, D:].rearrange("p n one -> p (n one)"), pools["ones"][:]
        )
        nc.vector.tensor_copy(self.v1[:, :, :D], self.v_nat[:])

    def stage_bounce(self):
        nc, pools, h = self.ctx["nc"], self.ctx, self.h
        dp = pools["dram_pool"]
        self.qdr = dp.tile([S, 128], F16, tag="qdr", name=f"qdr{h}")
        self.kdr = dp.tile([S, 128], F16, tag="kdr", name=f"kdr{h}")
        for dr, st16 in ((self.kdr, self.k16), (self.qdr, self.q16)):
            nc.gpsimd.dma_start(
                dr[:].rearrange("(n p) c -> p n c", p=128), st16[:]
            )

    def stage_xbar(self):
        nc, tc = self.ctx["nc"], self.ctx["tc"]
        with tc.high_priority():
            nc.sync.dma_start_transpose(self.kT[:], self.kdr[:])
            nc.sync.dma_start_transpose(self.qT[:], self.qdr[:])

    # ---- head 0: chunked ramp + JIT PE transposes ----
    def ramp_dma(self):
        nc, pools = self.ctx["nc"], self.ctx
        self._alloc()
        # zero qT/kT rows 64..127 (gpsimd, idle during the ramp): scores
        # contract K=128 against zero-padded weights because half-height
        # (K=64) matmuls only light up half the PE array and the HAM
        # clock-gate then never opens (1.2 GHz).
        nc.gpsimd.memset(self.qT[D:, :], 0.0)
        nc.gpsimd.memset(self.kT[D:, :], 0.0)
        chunks = ((0, 4), (4, 12))  # n-block ranges: small first chunk
        for n0, nn in chunks:
            ns = slice(n0, n0 + nn)
            for nat, st16, src in (
                (self.k_nat, self.k16, pools["k_dram"]),
                (self.q_nat, self.q16, pools["q_dram"]),
            ):
                nc.sync.dma_start(
                    nat[:, ns, :],
                    src[0].rearrange("(n p) d -> p n d", p=128)[:, ns, :],
                )
                nc.vector.tensor_copy(st16[:, ns, :D], nat[:, ns, :])
            if n0 == 0:
                nc.sync.dma_start(
                    self.v_nat[:],
                    pools["v_dram"][0].rearrange("(n p) d -> p n d", p=128),
                )
        nc.vector.tensor_copy(
            self.v1[:, :, D:].rearrange("p n one -> p (n one)"), pools["ones"][:]
        )
        nc.vector.tensor_copy(self.v1[:, :, :D], self.v_nat[:])
        # pre-transpose only what group 0 needs; the rest JIT lazily
        for n in (0, 1, 2):
            self.ensure_h0("k", n)
        for n in (0, 1, 2, 3):
            self.ensure_h0("q", n)

    def ensure_h0(self, kind, n):
        """JIT a [64, 128] PE transpose of staging block n into qT/kT."""
        if n in self.ready[kind]:
            return
        self.ready[kind].add(n)
        nc, pools = self.ctx["nc"], self.ctx
        st16, tT = (self.q16, self.qT) if kind == "q" else (self.k16, self.kT)
        tp = pools["ps_o_pool"].tile([D, 128], F16, tag="oT", name="tp")
        nc.tensor.transpose(tp[:], st16[:, n, :D], pools["ident16"][:])
        nc.vector.tensor_copy(tT[:D, n * 128:(n + 1) * 128], tp[:])


def _attention(tc):
    nc = tc.nc
    q_dram = nc.dram_tensor("query", [HPC, S, D], F32, kind="ExternalInput").ap()
    k_dram = nc.dram_tensor("key", [HPC, S, D], F32, kind="ExternalInput").ap()
    v_dram = nc.dram_tensor("value", [HPC, S, D], F32, kind="ExternalInput").ap()
    o_dram = nc.dram_tensor("out", [HPC, S, D], F32, kind="ExternalOutput").ap()

    with (
        tc.tile_pool(name="const", bufs=1) as const_pool,
        tc.tile_pool(name="head_io", bufs=4) as head_pool,
        tc.tile_pool(name="qkt", bufs=4) as qkt_pool,
        tc.tile_pool(name="et", bufs=3) as et_pool,
        tc.tile_pool(name="epi", bufs=4) as epi_pool,
        tc.tile_pool(name="dram", bufs=4, space="DRAM") as dram_pool,
        tc.tile_pool(name="ps_g", bufs=2, space="PSUM") as ps_g_pool,
        tc.tile_pool(name="ps_o", bufs=2, space="PSUM") as ps_o_pool,
    ):
        ones = const_pool.tile([128, NK], F16)
        nc.vector.memset(ones[:], 1.0)
        # pre-warm the exp table so ACT_TABLE_LOAD overlaps the DMA ramp
        warm = const_pool.tile([128, 1], F16)
        nc.vector.memset(warm[:], 0.0)
        nc.scalar.activation(warm[:], warm[:], EXP)
        ident16 = const_pool.tile([128, 128], F16)
        make_identity(nc, ident16[:])

        ctx = {
            "nc": nc, "tc": tc, "q_dram": q_dram, "k_dram": k_dram, "v_dram": v_dram,
            "head_pool": head_pool, "qkt_pool": qkt_pool,
            "dram_pool": dram_pool, "ps_o_pool": ps_o_pool,
            "ones": ones, "ident16": ident16,
        }

        # HAM warm-up: ~6.8us of chained dummy matmuls while the DMA ramp
        # runs. The PE clock-gate only opens (1.2 -> 2.4 GHz) after a
        # fully-busy free-running 3.4us window (worst case ~7us of
        # continuous activity), and the steady-state pipeline's micro-idles
        # never provide one; without this the whole kernel runs at half PE
        # clock and the PE becomes the bottleneck. The accumulation chain
        # keeps the Tile scheduler from scattering the matmuls.
        dummy = const_pool.tile([128, 640], F16)
        nc.vector.memset(dummy[:], 0.0)
        warm_ps = ps_g_pool.tile([128, GRP, QB], F32, tag="s", name="warm_ps")
        for i in range(6):
            nc.tensor.matmul(
                warm_ps[:, 0, :], dummy[:, :128], dummy[:, 128:],
                start=(i == 0), stop=(i == 5),
            )

        heads = [_HeadInputs(ctx, h) for h in range(HPC)]
        heads[0].ramp_dma()
        for hd in heads[1:]:
            hd._alloc()
        for hd in heads[1:]:
            # start ALL heads' input loads first (transfers overlap the
            # pad memsets), all on the idle gpsimd during the ramp
            hd.stage_nat()
        for hd in heads[1:]:
            nc.gpsimd.memset(hd.q16[:, :, D:], 0.0)
            nc.gpsimd.memset(hd.k16[:, :, D:], 0.0)

        # deferred-action scheduler, keyed by unit index
        actions = []
        aseq = [0]

        def defer(due_u, fn):
            heapq.heappush(actions, (due_u, aseq[0], fn))
            aseq[0] += 1

        def run_due(u):
            while actions and actions[0][0] <= u:
                heapq.heappop(actions)[2]()

        def emit_scores(units):
            slot = ps_g_pool.tile([128, GRP, QB], F32, tag="s", name="s")
            for j, u in enumerate(units):
                p, k = divmod(u, NK)
                h, qb = divmod(p, NQB)
                hd = heads[h]
                if h == 0:
                    hd.ensure_h0("k", k)
                    for nb in range(qb * 4, qb * 4 + 4):
                        hd.ensure_h0("q", nb)
                nc.tensor.matmul(
                    slot[:, j, :],
                    hd.kT[:, k * 128:(k + 1) * 128],
                    hd.qT[:, qb * QB:(qb + 1) * QB],
                    start=True, stop=True,
                )
            return slot

        def emit_epilogue(h, qb, oT16, u_end):
            """Normalize + un-transpose oT16 [65,512] -> out [512,64] using
            PE transposes (4x ~215ns fits in the PE's per-group slack): no
            DRAM round trip, so no DMA lateness can ever reach the critical
            DVE stream. The single 4-wide tp allocation keeps the ps_o ring
            parity even (oT and tp alternate slots)."""
            o_sb = epi_pool.tile([128, 4, D], F32, tag="o_sb", name="o_sb")
            tp_ref = []

            def stage_tp():
                tp = ps_o_pool.tile([128, 4, 68], F16, tag="oT", name="tp_epi")
                for j in range(4):
                    nc.tensor.transpose(
                        tp[:, j, :65], oT16[:, j * 128:(j + 1) * 128],
                        ident16[:65, :65],
                    )
                tp_ref.append(tp)

            def stage_d():
                tp = tp_ref[0]
                den = epi_pool.tile([128, 4], F32, tag="den", name="den")
                nc.vector.tensor_scalar_add(den[:], tp[:, :, D], 1.0)
                rec = epi_pool.tile([128, 4], F32, tag="rec", name="rec")
                nc.vector.reciprocal(rec[:], den[:])
                for j in range(4):
                    nc.vector.tensor_scalar_mul(
                        o_sb[:, j, :], tp[:, j, :D], rec[:, j:j + 1]
                    )

            def stage_e():
                nc.gpsimd.dma_start(
                    o_dram[h].rearrange("(n p) d -> p n d", p=128)[:, qb * 4:qb * 4 + 4, :],
                    o_sb[:],
                )

            defer(u_end + 2, stage_tp)
            defer(u_end + 4, stage_d)
            defer(u_end + 6, stage_e)

        # stage heads 1..3 early with wide margins: pools are deep enough
        # (bufs=4) that no slot WAR exists; loads already run from the ramp
        for h_n, base in ((1, 18), (2, 34), (3, 50)):
            defer(base + 0, heads[h_n].stage_cast)
            defer(base + 2, heads[h_n].stage_bounce)
            defer(base + 6, heads[h_n].stage_xbar)
        # pre-spread head-0's q-block JITs into PE slack ahead of each
        # pass boundary instead of a 4-wide burst at the boundary itself
        for qn, qbase in ((4, 8), (8, 24), (12, 40)):
            for i in range(4):
                defer(
                    qbase + i,
                    (lambda nn: lambda: heads[0].ensure_h0("q", nn))(qn + i),
                )

        groups = [
            list(range(gs, min(gs + GRP, UNITS))) for gs in range(0, UNITS, GRP)
        ]
        slot_cur = emit_scores(groups[0])
        oT = None
        for g, units in enumerate(groups):
            w = len(units)
            et = et_pool.tile([128, GRP, QB], F16, tag="et", name="et")
            nc.scalar.activation(
                et[:, :w, :], slot_cur[:, :w, :], EXP, scale=SCALE
            )
            if g + 1 < len(groups):
                slot_cur = emit_scores(groups[g + 1])
            for j, u in enumerate(units):
                run_due(u)
                p, k = divmod(u, NK)
                h, qb = divmod(p, NQB)
                if k == 0:
                    oT = ps_o_pool.tile([65, QB], F32, tag="oT", name="oT")
                nc.tensor.matmul(
                    oT[:], heads[h].v1[:, k, :], et[:, j, :],
                    start=(k == 0), stop=(k == NK - 1),
                )
                if k == NK - 1:
                    # evict oT immediately (fp16 cast on DVE): the next
                    # pass's oT shares the slot ping-pong and its first PV
                    # must not wait
                    oT16 = epi_pool.tile([65, QB], F16, tag="oT16", name="oT16")
                    nc.vector.tensor_copy(oT16[:], oT[:])
                    emit_epilogue(h, qb, oT16, u)
        while actions:
            heapq.heappop(actions)[2]()


_NC_CACHE = None
_TRACE_READY = False


def _enable_tracing():
    """Register the NTFF profile hook that this image's antenv lacks, and
    keep profiling artifacts local instead of uploading to a bucket."""
    global _TRACE_READY
    if _TRACE_READY:
        return
    import sys
    import types

    import antenv
    import concourse.bass_utils as bu
    from trn_agent_boot.trn_boot import _ntff_profile_via_ctypes

    if "antenv.axon_hooks" not in sys.modules:
        mod = types.ModuleType("antenv.axon_hooks")
        mod._hook = None

        def set_axon_ntff_profile_hook(h):
            mod._hook = h

        def get_axon_ntff_profile_hook():
            return mod._hook

        mod.set_axon_ntff_profile_hook = set_axon_ntff_profile_hook
        mod.get_axon_ntff_profile_hook = get_axon_ntff_profile_hook
        sys.modules["antenv.axon_hooks"] = mod
        antenv.axon_hooks = mod

    hooks = sys.modules["antenv.axon_hooks"]
    if hooks.get_axon_ntff_profile_hook() is None:
        hooks.set_axon_ntff_profile_hook(
            _ntff_profile_via_ctypes("/opt/axon/libaxon_pjrt.so")
        )
    bu.upload_artifacts = lambda tmpdir: tmpdir
    _TRACE_READY = True


def _build():
    global _NC_CACHE
    if _NC_CACHE is None:
        nc = bacc.Bacc("TRN2", target_bir_lowering=False, debug=False)
        with tile.TileContext(nc) as tc:
            _attention(tc)
        nc.compile()
        _NC_CACHE = nc
    return _NC_CACHE


def _run(query, key, value, trace=False, tmpdir=None):
    if trace:
        _enable_tracing()
    q = np.ascontiguousarray(np.asarray(query, dtype=np.float32).reshape(B * H, S, D))
    k = np.ascontiguousarray(np.asarray(key, dtype=np.float32).reshape(B * H, S, D))
    v = np.ascontiguousarray(np.asarray(value, dtype=np.float32).reshape(B * H, S, D))
    in_maps = [
        {
            "query": q[c * HPC:(c + 1) * HPC],
            "key": k[c * HPC:(c + 1) * HPC],
            "value": v[c * HPC:(c + 1) * HPC],
        }
        for c in range(N_CORES)
    ]
    nc = _build()
    res = run_bass_kernel_spmd(
        nc, in_maps, core_ids=list(range(N_CORES)), trace=trace, tmpdir=tmpdir
    )
    out = np.stack([res.results[c]["out"] for c in range(N_CORES)])  # [8, HPC, S, D]
    return out.reshape(B, H, S, D), res


def kernel(query, key, value):
    out, _ = _run(query, key, value, trace=bool(int(os.environ.get("BASS_TRACE", "0"))))
    return out


# revision 29
# speedup vs baseline: 1.1801x; 1.0752x over previous
"""Multi-head attention with "restricted softmax" on 8 TRN2 NeuronCores.

Reference computation (per head):
    score = Q @ K.T / sqrt(D)                       # [S, S]
    attn  = exp(score) / (1 + sum_k exp(score))     # restricted softmax
            (mathematically identical to the max-clamped reference form)
    out   = attn @ V                                # [S, D]

Full problem: B=2, H=16, S=2048, D=64  ->  32 heads, 4 heads per core.

The ScalarEngine's exp is the hard floor (1 elem/cycle/lane @ 1.2 GHz,
(N+352)/1.2 ns per instruction), so everything is built to keep it
saturated with the widest ACTIVATEs PSUM allows:
  - Scores computed TRANSPOSED (S^T[k, q]) in [128, 512] units; THREE
    units share one ACTIVATE (N=1536 -> 1.025 ns/elem vs 1.12 at N=1024):
    6 PSUM banks double-buffered + 2 banks for the PV accumulator ring.
  - Scores matmuls contract K=128 with zero-padded rows 64..127: K=64
    streams at the same rate, but half-height weights only light up half
    the PE array and the HAM clock-gate then never opens (kernel stuck at
    1.2 GHz instead of 2.4). A chained dummy-matmul burst during the DMA
    ramp opens the gate before the pipeline starts.
  - PV uses lhsT=[V | 1] so PSUM row 64 accumulates sum_k exp (the
    softmax denominator) for free.
  - The epilogue (normalize + [d,q]->[q,d]) uses PE transposes (4 x
    ~215ns/pass fits the PE's per-group slack) instead of a DRAM round
    trip: DMA lateness can then never reach the critical DVE stream. The
    oT eviction cast is issued immediately at pass end; the one 4-wide
    tp allocation per pass keeps the ps_o ring parity even.
  - All emission is deferred via a unit-indexed action heap so every op's
    dependency is already satisfied at dispatch: the DVE / Sync / GpSimd
    queues are in-order and a waiting op head-of-line-blocks its stream.
  - Queue split: GpSimd carries staging loads/bounces + output DMAs,
    Sync carries only the staging X-bar transposes (HWDGE-only). Head
    staging is fully WAR-free (bufs=4 pools) and starts during the ramp.
  - Head 0 is staged by PE JIT transposes (the PE is idle in the ramp);
    heads 1..3 go through an fp16 DRAM bounce + X-bar transpose.
  - The exp table is pre-warmed so ACT_TABLE_LOAD overlaps the ramp.
"""

import heapq
import os

import numpy as np

import concourse.bass as bass  # noqa: F401  (bass must import before tile)
import concourse.mybir as mybir
import concourse.tile as tile
from concourse import bacc
from concourse.bass_utils import run_bass_kernel_spmd
from concourse.masks import make_identity

B, H, S, D = 2, 16, 2048, 64
N_CORES = 8
HPC = (B * H) // N_CORES  # heads per core = 4

F32 = mybir.dt.float32
F16 = mybir.dt.float16
EXP = mybir.ActivationFunctionType.Exp

SCALE = 1.0 / 8.0   # 1/sqrt(D)
NK = S // 128       # 16 k-tiles of 128
QB = 512            # q-block width per pass
NQB = S // QB       # 4 q-blocks per head
NPASS = HPC * NQB   # 16 passes
UNITS = NPASS * NK  # 256 scores units of [128k, 512q]
GRP = 3             # units per ACTIVATE group


class _HeadInputs:
    """Per-head staged inputs: fp16 Q^T/K^T [128, S] (rows 64..127 are
    zeros; scores contract K=128 to keep the PE array fully lit for the
    HAM activity monitor) and [V | 1] fp16.

    Heads 1..3: DMA X-bar transpose of an fp16 bounce buffer in DRAM
    (zero PE cost), with each stage emitted as a deferred action so no
    queue ever dispatches a waiting op. Head 0: PE transposes JIT'd into
    the idle ramp."""

    def __init__(self, ctx, h):
        self.ctx = ctx
        self.h = h
        self.ready = {"q": set(), "k": set()}  # head-0 JIT transpose state

    def _alloc(self):
        pools, h = self.ctx, self.h
        hp = pools["head_pool"]
        self.q_nat = hp.tile([128, NK, D], F32, tag="q_nat", name=f"q_nat{h}")
        self.k_nat = hp.tile([128, NK, D], F32, tag="k_nat", name=f"k_nat{h}")
        self.v_nat = hp.tile([128, NK, D], F32, tag="v_nat", name=f"v_nat{h}")
        # fp16 staging; cols 64..127 are never written nor read
        self.q16 = hp.tile([128, NK, 128], F16, tag="q16", name=f"q16_{h}")
        self.k16 = hp.tile([128, NK, 128], F16, tag="k16", name=f"k16_{h}")
        self.v1 = hp.tile([128, NK, D + 1], F16, tag="v1", name=f"v1_{h}")
        self.qT = pools["qkt_pool"].tile([128, S], F16, tag="qT", name=f"qT{h}")
        self.kT = pools["qkt_pool"].tile([128, S], F16, tag="kT", name=f"kT{h}")

    # ---- heads 1..3: four deferred stages ----
    def stage_nat(self):
        nc, pools = self.ctx["nc"], self.ctx
        for nat, src in (
            (self.k_nat, pools["k_dram"]), (self.q_nat, pools["q_dram"]),
            (self.v_nat, pools["v_dram"]),
        ):
            nc.gpsimd.dma_start(
                nat[:], src[self.h].rearrange("(n p) d -> p n d", p=128)
            )

    def stage_cast(self):
        nc, pools = self.ctx["nc"], self.ctx
        nc.vector.tensor_copy(self.k16[:, :, :D], self.k_nat[:])
        nc.vector.tensor_copy(self.q16[:, :, :D], self.q_nat[:])
        nc.vector.tensor_copy(
            self.v1[:, :, D:].rearrange("p n one -> p (n one)"), pools["ones"][:]
        )
        nc.vector.tensor_copy(self.v1[:, :, :D], self.v_nat[:])

    def stage_bounce(self):
        nc, pools, h = self.ctx["nc"], self.ctx, self.h
        dp = pools["dram_pool"]
        self.qdr = dp.tile([S, 128], F16, tag="qdr", name=f"qdr{h}")
        self.kdr = dp.tile([S, 128], F16, tag="kdr", name=f"kdr{h}")
        for dr, st16 in ((self.kdr, self.k16), (self.qdr, self.q16)):
            nc.gpsimd.dma_start(
                dr[:].rearrange("(n p) c -> p n c", p=128), st16[:]
            )

    def stage_xbar(self):
        nc, tc = self.ctx["nc"], self.ctx["tc"]
        with tc.high_priority():
            nc.sync.dma_start_transpose(self.kT[:], self.kdr[:])
            nc.sync.dma_start_transpose(self.qT[:], self.qdr[:])

    # ---- head 0: chunked ramp + JIT PE transposes ----
    def ramp_dma(self):
        nc, pools = self.ctx["nc"], self.ctx
        self._alloc()
        # zero qT/kT rows 64..127 (gpsimd, idle during the ramp): scores
        # contract K=128 against zero-padded weights because half-height
        # (K=64) matmuls only light up half the PE array and the HAM
        # clock-gate then never opens (1.2 GHz).
        nc.gpsimd.memset(self.qT[D:, :], 0.0)
        nc.gpsimd.memset(self.kT[D:, :], 0.0)
        chunks = ((0, 4), (4, 12))  # n-block ranges: small first chunk
        for n0, nn in chunks:
            ns = slice(n0, n0 + nn)
            for nat, st16, src in (
                (self.k_nat, self.k16, pools["k_dram"]),
                (self.q_nat, self.q16, pools["q_dram"]),
            ):
                nc.sync.dma_start(
                    nat[:, ns, :],
                    src[0].rearrange("(n p) d -> p n d", p=128)[:, ns, :],
                )
                nc.vector.tensor_copy(st16[:, ns, :D], nat[:, ns, :])
            if n0 == 0:
                nc.sync.dma_start(
                    self.v_nat[:],
                    pools["v_dram"][0].rearrange("(n p) d -> p n d", p=128),
                )
        nc.vector.tensor_copy(
            self.v1[:, :, D:].rearrange("p n one -> p (n one)"), pools["ones"][:]
        )
        nc.vector.tensor_copy(self.v1[:, :, :D], self.v_nat[:])
        # pre-transpose only what group 0 needs; the rest JIT lazily
        for n in (0, 1, 2):
            self.ensure_h0("k", n)
        for n in (0, 1, 2, 3):
            self.ensure_h0("q", n)

    def ensure_h0(self, kind, n):
        """JIT a [64, 128] PE transpose of staging block n into qT/kT."""
        if n in self.ready[kind]:
            return
        self.ready[kind].add(n)
        nc, pools = self.ctx["nc"], self.ctx
        st16, tT = (self.q16, self.qT) if kind == "q" else (self.k16, self.kT)
        tp = pools["ps_o_pool"].tile([D, 128], F16, tag="oT", name="tp")
        nc.tensor.transpose(tp[:], st16[:, n, :D], pools["ident16"][:])
        nc.vector.tensor_copy(tT[:D, n * 128:(n + 1) * 128], tp[:])


def _attention(tc):
    nc = tc.nc
    q_dram = nc.dram_tensor("query", [HPC, S, D], F32, kind="ExternalInput").ap()
    k_dram = nc.dram_tensor("key", [HPC, S, D], F32, kind="ExternalInput").ap()
    v_dram = nc.dram_tensor("value", [HPC, S, D], F32, kind="ExternalInput").ap()
    o_dram = nc.dram_tensor("out", [HPC, S, D], F32, kind="ExternalOutput").ap()

    with (
        tc.tile_pool(name="const", bufs=1) as const_pool,
        tc.tile_pool(name="head_io", bufs=4) as head_pool,
        tc.tile_pool(name="qkt", bufs=4) as qkt_pool,
        tc.tile_pool(name="et", bufs=3) as et_pool,
        tc.tile_pool(name="epi", bufs=4) as epi_pool,
        tc.tile_pool(name="dram", bufs=4, space="DRAM") as dram_pool,
        tc.tile_pool(name="ps_g", bufs=2, space="PSUM") as ps_g_pool,
        tc.tile_pool(name="ps_o", bufs=2, space="PSUM") as ps_o_pool,
    ):
        ones = const_pool.tile([128, NK], F16)
        nc.vector.memset(ones[:], 1.0)
        # pre-warm the exp table so ACT_TABLE_LOAD overlaps the DMA ramp
        warm = const_pool.tile([128, 1], F16)
        nc.vector.memset(warm[:], 0.0)
        nc.scalar.activation(warm[:], warm[:], EXP)
        ident16 = const_pool.tile([128, 128], F16)
        make_identity(nc, ident16[:])

        ctx = {
            "nc": nc, "tc": tc, "q_dram": q_dram, "k_dram": k_dram, "v_dram": v_dram,
            "head_pool": head_pool, "qkt_pool": qkt_pool,
            "dram_pool": dram_pool, "ps_o_pool": ps_o_pool,
            "ones": ones, "ident16": ident16,
        }

        # HAM warm-up: ~6.8us of chained dummy matmuls while the DMA ramp
        # runs. The PE clock-gate only opens (1.2 -> 2.4 GHz) after a
        # fully-busy free-running 3.4us window (worst case ~7us of
        # continuous activity), and the steady-state pipeline's micro-idles
        # never provide one; without this the whole kernel runs at half PE
        # clock and the PE becomes the bottleneck. The accumulation chain
        # keeps the Tile scheduler from scattering the matmuls.
        dummy = const_pool.tile([128, 640], F16)
        nc.vector.memset(dummy[:], 0.0)
        warm_ps = ps_g_pool.tile([128, GRP, QB], F32, tag="s", name="warm_ps")
        for i in range(12):
            nc.tensor.matmul(
                warm_ps[:, 0, :], dummy[:, :128], dummy[:, 128:],
                start=(i == 0), stop=(i == 11),
            )

        heads = [_HeadInputs(ctx, h) for h in range(HPC)]
        heads[0].ramp_dma()
        for hd in heads[1:]:
            hd._alloc()
        for hd in heads[1:]:
            # start ALL heads' input loads first (transfers overlap the
            # pad memsets), all on the idle gpsimd during the ramp
            hd.stage_nat()
        for hd in heads[1:]:
            nc.gpsimd.memset(hd.q16[:, :, D:], 0.0)
            nc.gpsimd.memset(hd.k16[:, :, D:], 0.0)

        # deferred-action scheduler, keyed by unit index
        actions = []
        aseq = [0]

        def defer(due_u, fn):
            heapq.heappush(actions, (due_u, aseq[0], fn))
            aseq[0] += 1

        def run_due(u):
            while actions and actions[0][0] <= u:
                heapq.heappop(actions)[2]()

        def emit_scores(units):
            slot = ps_g_pool.tile([128, GRP, QB], F32, tag="s", name="s")
            for j, u in enumerate(units):
                p, k = divmod(u, NK)
                h, qb = divmod(p, NQB)
                hd = heads[h]
                if h == 0:
                    hd.ensure_h0("k", k)
                    for nb in range(qb * 4, qb * 4 + 4):
                        hd.ensure_h0("q", nb)
                nc.tensor.matmul(
                    slot[:, j, :],
                    hd.kT[:, k * 128:(k + 1) * 128],
                    hd.qT[:, qb * QB:(qb + 1) * QB],
                    start=True, stop=True,
                )
            return slot

        def emit_epilogue(h, qb, oT16, u_end):
            """Normalize + un-transpose oT16 [65,512] -> out [512,64] using
            PE transposes (4x ~215ns fits in the PE's per-group slack): no
            DRAM round trip, so no DMA lateness can ever reach the critical
            DVE stream. The single 4-wide tp allocation keeps the ps_o ring
            parity even (oT and tp alternate slots)."""
            o_sb = epi_pool.tile([128, 4, D], F32, tag="o_sb", name="o_sb")
            tp_ref = []

            def stage_tp():
                tp = ps_o_pool.tile([128, 4, 68], F16, tag="oT", name="tp_epi")
                for j in range(4):
                    nc.tensor.transpose(
                        tp[:, j, :65], oT16[:, j * 128:(j + 1) * 128],
                        ident16[:65, :65],
                    )
                tp_ref.append(tp)

            def stage_d():
                tp = tp_ref[0]
                den = epi_pool.tile([128, 4], F32, tag="den", name="den")
                nc.vector.tensor_scalar_add(den[:], tp[:, :, D], 1.0)
                rec = epi_pool.tile([128, 4], F32, tag="rec", name="rec")
                nc.vector.reciprocal(rec[:], den[:])
                for j in range(4):
                    nc.vector.tensor_scalar_mul(
                        o_sb[:, j, :], tp[:, j, :D], rec[:, j:j + 1]
                    )

            def stage_e():
                nc.gpsimd.dma_start(
                    o_dram[h].rearrange("(n p) d -> p n d", p=128)[:, qb * 4:qb * 4 + 4, :],
                    o_sb[:],
                )

            defer(u_end + 2, stage_tp)
            defer(u_end + 4, stage_d)
            defer(u_end + 6, stage_e)

        # stage heads 1..3 early with wide margins: pools are deep enough
        # (bufs=4) that no slot WAR exists; loads already run from the ramp
        for h_n, base in ((1, 18), (2, 34), (3, 50)):
            defer(base + 0, heads[h_n].stage_cast)
            defer(base + 2, heads[h_n].stage_bounce)
            defer(base + 6, heads[h_n].stage_xbar)
        # pre-spread head-0's q-block JITs into PE slack ahead of each
        # pass boundary instead of a 4-wide burst at the boundary itself
        for qn, qbase in ((4, 8), (8, 24), (12, 40)):
            for i in range(4):
                defer(
                    qbase + i,
                    (lambda nn: lambda: heads[0].ensure_h0("q", nn))(qn + i),
                )

        groups = [
            list(range(gs, min(gs + GRP, UNITS))) for gs in range(0, UNITS, GRP)
        ]
        slot_cur = emit_scores(groups[0])
        oT = None
        for g, units in enumerate(groups):
            w = len(units)
            et = et_pool.tile([128, GRP, QB], F16, tag="et", name="et")
            nc.scalar.activation(
                et[:, :w, :], slot_cur[:, :w, :], EXP, scale=SCALE
            )
            if g + 1 < len(groups):
                slot_cur = emit_scores(groups[g + 1])
            for j, u in enumerate(units):
                run_due(u)
                p, k = divmod(u, NK)
                h, qb = divmod(p, NQB)
                if k == 0:
                    oT = ps_o_pool.tile([65, QB], F32, tag="oT", name="oT")
                nc.tensor.matmul(
                    oT[:], heads[h].v1[:, k, :], et[:, j, :],
                    start=(k == 0), stop=(k == NK - 1),
                )
                if k == NK - 1:
                    # evict oT immediately (fp16 cast on DVE): the next
                    # pass's oT shares the slot ping-pong and its first PV
                    # must not wait
                    oT16 = epi_pool.tile([65, QB], F16, tag="oT16", name="oT16")
                    nc.vector.tensor_copy(oT16[:], oT[:])
                    emit_epilogue(h, qb, oT16, u)
        while actions:
            heapq.heappop(actions)[2]()


_NC_CACHE = None
_TRACE_READY = False


def _enable_tracing():
    """Register the NTFF profile hook that this image's antenv lacks, and
    keep profiling artifacts local instead of uploading to a bucket."""
    global _TRACE_READY
    if _TRACE_READY:
        return
    import sys
    import types

    import antenv
    import concourse.bass_utils as bu
    from trn_agent_boot.trn_boot import _ntff_profile_via_ctypes

    if "antenv.axon_hooks" not in sys.modules:
        mod = types.ModuleType("antenv.axon_hooks")
        mod._hook = None

        def set_axon_ntff_profile_hook(h):
            mod._hook = h

        def get_axon_ntff_profile_hook():
            return mod._hook

        mod.set_axon_ntff_profile_hook = set_axon_ntff_profile_hook
        mod.get_axon_ntff_profile_hook = get_axon_ntff_profile_hook
        sys.modules["antenv.axon_hooks"] = mod
        antenv.axon_hooks = mod

    hooks = sys.modules["antenv.axon_hooks"]
    if hooks.get_axon_ntff_profile_hook() is None:
        hooks.set_axon_ntff_profile_hook(
            _ntff_profile_via_ctypes("/opt/axon/libaxon_pjrt.so")
        )
    bu.upload_artifacts = lambda tmpdir: tmpdir
    _TRACE_READY = True


def _build():
    global _NC_CACHE
    if _NC_CACHE is None:
        nc = bacc.Bacc("TRN2", target_bir_lowering=False, debug=False)
        with tile.TileContext(nc) as tc:
            _attention(tc)
        nc.compile()
        _NC_CACHE = nc
    return _NC_CACHE


def _run(query, key, value, trace=False, tmpdir=None):
    if trace:
        _enable_tracing()
    q = np.ascontiguousarray(np.asarray(query, dtype=np.float32).reshape(B * H, S, D))
    k = np.ascontiguousarray(np.asarray(key, dtype=np.float32).reshape(B * H, S, D))
    v = np.ascontiguousarray(np.asarray(value, dtype=np.float32).reshape(B * H, S, D))
    in_maps = [
        {
            "query": q[c * HPC:(c + 1) * HPC],
            "key": k[c * HPC:(c + 1) * HPC],
            "value": v[c * HPC:(c + 1) * HPC],
        }
        for c in range(N_CORES)
    ]
    nc = _build()
    res = run_bass_kernel_spmd(
        nc, in_maps, core_ids=list(range(N_CORES)), trace=trace, tmpdir=tmpdir
    )
    out = np.stack([res.results[c]["out"] for c in range(N_CORES)])  # [8, HPC, S, D]
    return out.reshape(B, H, S, D), res


def kernel(query, key, value):
    out, _ = _run(query, key, value, trace=bool(int(os.environ.get("BASS_TRACE", "0"))))
    return out
